# revision 1
# baseline (speedup 1.0000x reference)
"""Multihead attention (B=2, S=2048, D=1024, 16 heads) on 8 trn2 NeuronCores.

Sharding: data-parallel over batch (2 groups of 4 cores), tensor-parallel over
heads within a group (4 heads/core, W_q/W_k/W_v column-sliced, W_o row-sliced).
Each core returns a partial [2048, 1024] output; the host sums the 4 partials
per batch and adds the constant row bv @ Wo + bo (the V-bias contribution is
constant because softmax rows sum to 1).

Device-side dataflow per core (all matmuls in float32r — full PE rate):
  xT/kvT arrive pre-transposed [D, S] from the host so projections contract
  over D on partitions.  Q^T,K^T are computed head-major [256, S]; V natural
  [S, 256] with a trailing per-head column holding the 0/1 key mask.  The
  key-padding mask is applied MULTIPLICATIVELY on V rows (exp(S+m*-inf) ==
  exp(S)*m01), which keeps the exp on ScalarE bias-free so it can batch
  across PSUM banks.  S^T_h = K_h^T.T @ Q_h^T per kv-tile, exp into P^T,
  then P^T V accumulates over kv-tiles; the mask column yields the softmax
  denominators in PSUM row 64.  Output projection is interleaved per q-chunk.
"""

import numpy as np

import concourse.bacc as bacc
import concourse.tile as tile
import concourse.mybir as mybir
from concourse.bass_utils import run_bass_kernel_spmd

F32 = mybir.dt.float32
F32R = mybir.dt.float32r
EXP = mybir.ActivationFunctionType.Exp
MULT = mybir.AluOpType.mult

B, SQ, SKV = 2, 2048, 2048
D, NH, HD = 1024, 16, 64
NCORES = 8
HPC = NH // (NCORES // B)     # 4 heads per core
CS = HPC * HD                 # 256 projection columns per core
NKT = SKV // 128              # 16 kv tiles
QC = 512                      # q chunk
NQC = SQ // QC                # 4 q chunks
NDT = D // 128                # 8 contraction tiles
NST = SQ // 128               # 16 output row tiles


def _build(loop_n: int = 1, variant: str = "unpaired_g2"):
    nc = bacc.Bacc(None, target_bir_lowering=False)
    xT = nc.dram_tensor("xT", [D, SQ], F32R, kind="ExternalInput")
    kvT = nc.dram_tensor("kvT", [D, SKV], F32R, kind="ExternalInput")
    wq = nc.dram_tensor("wq", [128, NDT, CS], F32R, kind="ExternalInput")
    wk = nc.dram_tensor("wk", [128, NDT, CS], F32R, kind="ExternalInput")
    wv = nc.dram_tensor("wv", [128, NDT, CS], F32R, kind="ExternalInput")
    wo = nc.dram_tensor("wo", [128, 2, D], F32R, kind="ExternalInput")
    bqk = nc.dram_tensor("bqk", [128, 4], F32, kind="ExternalInput")
    mcol = nc.dram_tensor("mcol", [128, NKT], F32, kind="ExternalInput")
    mones = nc.dram_tensor("mones", [128, NKT, HPC], F32R, kind="ExternalInput")
    out_p = nc.dram_tensor("out_p", [SQ, D], F32, kind="ExternalOutput")

    with tile.TileContext(nc) as tc:
        with tc.tile_pool(name="const", bufs=1) as const, \
             tc.tile_pool(name="big", bufs=1) as big:
            wq_sb = const.tile([128, NDT, CS], F32R)
            wk_sb = const.tile([128, NDT, CS], F32R)
            wv_sb = const.tile([128, NDT, CS], F32R)
            wo_sb = const.tile([128, 2, D], F32R)
            bqk_sb = const.tile([128, 4], F32)
            mcol_sb = const.tile([128, NKT], F32)
            nc.gpsimd.dma_start(out=wq_sb, in_=wq[:, :, :])
            nc.gpsimd.dma_start(out=bqk_sb, in_=bqk[:, :])
            nc.gpsimd.dma_start(out=wk_sb, in_=wk[:, :, :])
            nc.gpsimd.dma_start(out=wv_sb, in_=wv[:, :, :])
            nc.gpsimd.dma_start(out=mcol_sb, in_=mcol[:, :])
            nc.gpsimd.dma_start(out=wo_sb, in_=wo[:, :, :])

            QT = big.tile([128, 2, SQ], F32R)        # [hd(2x128), q]
            KT = big.tile([128, 2, SKV], F32R)       # [hd(2x128), kv]
            V = big.tile([128, NKT, HPC, HD + 1], F32R)  # V in 0:64, mask col at 64
            OT = big.tile([128, 2, SQ], F32R)        # [c(2x128), q]

            if loop_n > 1:
                loop_cm = tc.For_i(0, loop_n, 1)
                loop_cm.__enter__()

            nc.gpsimd.dma_start(out=V[:, :, :, HD:HD + 1], in_=mones[:, :, :])

            # ---- Phase 1: projections ----
            # xT streams per-dq-tile (SP ring); kvT loads resident (ACT ring).
            with tc.tile_pool(name="xin", bufs=3) as xin, \
                 tc.tile_pool(name="kvin", bufs=1) as kvin:
                kvts = []
                for dt in range(NDT):
                    kvt_t = kvin.tile([128, SKV], F32R, tag=f"kv{dt}", name=f"kvt{dt}")
                    nc.scalar.dma_start(out=kvt_t, in_=kvT[dt * 128:(dt + 1) * 128, :])
                    kvts.append(kvt_t)

                with tc.tile_pool(name="pqk", bufs=1, space="PSUM") as pqk:
                    # Q^T: dq-tile-outer accumulation into 8 resident psum banks
                    psq = [pqk.tile([128, QC], F32, tag=f"pq{i}", name=f"psq{i}")
                           for i in range(8)]
                    for dt in range(NDT):
                        xt_t = xin.tile([128, SQ], F32R, tag="xt", name=f"xt{dt}")
                        nc.sync.dma_start(out=xt_t, in_=xT[dt * 128:(dt + 1) * 128, :])
                        for i in range(8):
                            mh, qc = i // NQC, i % NQC
                            nc.tensor.matmul(psq[i],
                                             wq_sb[:, dt, mh * 128:(mh + 1) * 128],
                                             xt_t[:, qc * QC:(qc + 1) * QC],
                                             start=(dt == 0), stop=(dt == NDT - 1))
                    for i in range(8):
                        mh, qc = i // NQC, i % NQC
                        nc.vector.tensor_scalar_add(out=QT[:, mh, qc * QC:(qc + 1) * QC],
                                                    in0=psq[i], scalar1=bqk_sb[:, mh:mh + 1])
                    # K^T: same structure over resident kvT tiles
                    psk = [pqk.tile([128, QC], F32, tag=f"pq{i}", name=f"psk{i}")
                           for i in range(8)]
                    for dt in range(NDT):
                        for i in range(8):
                            mh, qc = i // NQC, i % NQC
                            nc.tensor.matmul(psk[i],
                                             wk_sb[:, dt, mh * 128:(mh + 1) * 128],
                                             kvts[dt][:, qc * QC:(qc + 1) * QC],
                                             start=(dt == 0), stop=(dt == NDT - 1))
                    for i in range(8):
                        mh, qc = i // NQC, i % NQC
                        nc.vector.tensor_scalar_add(out=KT[:, mh, qc * QC:(qc + 1) * QC],
                                                    in0=psk[i], scalar1=bqk_sb[:, 2 + mh:3 + mh])

                with tc.tile_pool(name="pv", bufs=4, space="PSUM") as pv:
                    # V natural [kv, 256], masked rows zeroed (incl. mask col via DMA above)
                    for t in range(NKT):
                        ps = pv.tile([128, CS], F32, tag="pv")
                        for dt in range(NDT):
                            nc.tensor.matmul(ps,
                                             kvts[dt][:, t * 128:(t + 1) * 128],
                                             wv_sb[:, dt, :],
                                             start=(dt == 0), stop=(dt == NDT - 1))
                        nc.vector.tensor_scalar(out=V[:, t, :, 0:HD],
                                                in0=ps.rearrange("p (h d) -> p h d", h=HPC),
                                                scalar1=mcol_sb[:, t:t + 1], scalar2=None,
                                                op0=MULT)

            # ---- Phase 2: attention with interleaved output projection ----
            # Heads are processed in pairs occupying PE row groups 0-63 /
            # 64-127 so the K=64 S^T matmuls and the K=64 PV half-tiles run
            # concurrently in the two array halves (2x row tiling).
            interleave_op = variant != "paired_g3_post"
            if variant == "unpaired_g2":
                psc_cfg, pso_b, pout_b = ("ss2", 2), 2, 2
            elif variant == "unpaired_g3":
                psc_cfg, pso_b, pout_b = ("ss2", 2), 1, 1
            elif variant == "paired_g3":
                psc_cfg, pso_b, pout_b = ("ss11", 1), 1, 1
            else:  # paired_g3_post
                psc_cfg, pso_b, pout_b = ("ss11", 1), 2, 2

            from contextlib import ExitStack
            _ph2 = ExitStack()
            with _ph2:
                pp = _ph2.enter_context(tc.tile_pool(name="pp", bufs=1))
                outp = _ph2.enter_context(tc.tile_pool(name="outp", bufs=2))
                small = _ph2.enter_context(tc.tile_pool(name="small", bufs=2))
                _att = ExitStack()
                psc = _att.enter_context(tc.tile_pool(name="psc", bufs=1, space="PSUM"))
                pso = _att.enter_context(tc.tile_pool(name="pso", bufs=pso_b, space="PSUM"))
                pout = None
                if interleave_op:
                    pout = _att.enter_context(
                        tc.tile_pool(name="pout", bufs=pout_b, space="PSUM"))

                def norm_store(po_, po, mh, qsl):
                    rec = small.tile([HD + 1, QC], F32, tag="rec", name="rec")
                    nc.vector.reciprocal(out=rec[HD:HD + 1, :], in_=po_[HD:HD + 1, :])
                    rec0 = small.tile([1, QC], F32, tag="rec0", name="rec0")
                    nc.sync.dma_start(out=rec0[0:1, :], in_=rec[HD:HD + 1, :])
                    rb = small.tile([HD, QC], F32, tag="rb", name="rb")
                    nc.gpsimd.partition_broadcast(rb, rec0[0:1, :])
                    ot_tmp = small.tile([HD, QC], F32R, tag="ott", name="ot_tmp")
                    nc.vector.tensor_mul(out=ot_tmp, in0=po_[0:HD, :], in1=rb)
                    nc.sync.dma_start(out=OT[po:po + 64, mh, qsl], in_=ot_tmp)

                def out_proj(st, pool):
                    ot_sb = outp.tile([128, D], F32, tag="osb", name="ot_sb")
                    for nk in range(2):
                        ps = pool.tile([128, 512], F32, tag="po2", name="ps_out")
                        for ct in range(2):
                            nc.tensor.matmul(ps,
                                             OT[:, ct, st * 128:(st + 1) * 128],
                                             wo_sb[:, ct, nk * 512:(nk + 1) * 512],
                                             start=(ct == 0), stop=(ct == 1))
                        nc.vector.tensor_copy(out=ot_sb[:, nk * 512:(nk + 1) * 512], in_=ps)
                    nc.sync.dma_start(out=out_p[st * 128:(st + 1) * 128, :], in_=ot_sb)

                def pv_head(h, P, mh, qsl):
                    po = (h % 2) * 64
                    po_ = pso.tile([HD + 1, QC], F32, tag="po", name="po_")
                    for t in range(NKT):
                        nc.tensor.matmul(po_, V[:, t, h, :], P[:, t, :],
                                         start=(t == 0), stop=(t == NKT - 1))
                    norm_store(po_, po, mh, qsl)

                GEXP = 2 if variant == "unpaired_g2" else 3
                groups = []
                t0 = 0
                while t0 < NKT:
                    groups.append((t0, min(GEXP, NKT - t0)))
                    t0 += GEXP

                for qc in range(NQC):
                    qsl = slice(qc * QC, (qc + 1) * QC)
                    if variant.startswith("unpaired"):
                        for h in range(HPC):
                            mh, po = h // 2, (h % 2) * 64
                            P = pp.tile([128, NKT, QC], F32R, tag="P0", name="P")
                            for g0, gn in groups:
                                ss = psc.tile([128, GEXP, QC], F32, tag=psc_cfg[0],
                                              bufs=psc_cfg[1], name="ss")
                                for tt in range(gn):
                                    t = g0 + tt
                                    nc.tensor.matmul(ss[:, tt, :],
                                                     KT[po:po + 64, mh, t * 128:(t + 1) * 128],
                                                     QT[po:po + 64, mh, qsl],
                                                     start=True, stop=True)
                                nc.scalar.activation(out=P[:, g0:g0 + gn, :],
                                                     in_=ss[:, 0:gn, :], func=EXP, scale=0.125)
                            pv_head(h, P, mh, qsl)
                    else:
                        for hp in range(HPC // 2):
                            mh = hp
                            h0, h1 = 2 * hp, 2 * hp + 1
                            P0 = pp.tile([128, NKT, QC], F32R, tag="P0", name="P0")
                            P1 = pp.tile([128, NKT, QC], F32R, tag="P1", name="P1")
                            for g0, gn in groups:
                                ss0 = psc.tile([128, GEXP, QC], F32, tag="ss0",
                                               bufs=1, name="ss0")
                                ss1 = psc.tile([128, GEXP, QC], F32, tag="ss1",
                                               bufs=1, name="ss1")
                                for tt in range(gn):
                                    t = g0 + tt
                                    nc.tensor.matmul(ss0[:, tt, :],
                                                     KT[0:64, mh, t * 128:(t + 1) * 128],
                                                     QT[0:64, mh, qsl],
                                                     start=True, stop=True)
                                    nc.tensor.matmul(ss1[:, tt, :],
                                                     KT[64:128, mh, t * 128:(t + 1) * 128],
                                                     QT[64:128, mh, qsl],
                                                     start=True, stop=True)
                                nc.scalar.activation(out=P0[:, g0:g0 + gn, :],
                                                     in_=ss0[:, 0:gn, :], func=EXP, scale=0.125)
                                nc.scalar.activation(out=P1[:, g0:g0 + gn, :],
                                                     in_=ss1[:, 0:gn, :], func=EXP, scale=0.125)
                            pv_head(h0, P0, mh, qsl)
                            pv_head(h1, P1, mh, qsl)
                    if interleave_op:
                        for st in range(qc * NQC, (qc + 1) * NQC):
                            out_proj(st, pout)
                _att.close()
                if not interleave_op:
                    with tc.tile_pool(name="pout", bufs=pout_b, space="PSUM") as pout2:
                        for st in range(NST):
                            out_proj(st, pout2)

            if loop_n > 1:
                loop_cm.__exit__(None, None, None)

    nc.compile()
    return nc


_NC = None


def _get_nc():
    global _NC
    if _NC is None:
        _NC = _build()
    return _NC


def _shard_inputs(query_input, key_value_input, key_padding_mask,
                  Wq, bq, Wk, bk, Wv, bv, Wo, bo):
    in_maps = []
    for c in range(NCORES):
        b, hg = c // (NCORES // B), c % (NCORES // B)
        cs = slice(hg * CS, (hg + 1) * CS)
        m01 = np.where(key_padding_mask[b], np.float32(0.0), np.float32(1.0))
        mcol = np.ascontiguousarray(m01.reshape(NKT, 128).T)          # [128, NKT]
        mones = np.ascontiguousarray(
            np.repeat(mcol[:, :, None], HPC, axis=2))                 # [128, NKT, HPC]
        in_maps.append({
            "xT": np.ascontiguousarray(query_input[b].T),
            "kvT": np.ascontiguousarray(key_value_input[b].T),
            "wq": np.ascontiguousarray(Wq[:, cs].reshape(NDT, 128, CS).transpose(1, 0, 2)),
            "wk": np.ascontiguousarray(Wk[:, cs].reshape(NDT, 128, CS).transpose(1, 0, 2)),
            "wv": np.ascontiguousarray(Wv[:, cs].reshape(NDT, 128, CS).transpose(1, 0, 2)),
            "wo": np.ascontiguousarray(Wo[cs, :].reshape(2, 128, D).transpose(1, 0, 2)),
            "bqk": np.ascontiguousarray(
                np.stack([bq[cs][:128], bq[cs][128:], bk[cs][:128], bk[cs][128:]], axis=1)),
            "mcol": mcol,
            "mones": mones,
        })
    return in_maps


def kernel(query_input, key_value_input, key_padding_mask,
           Wq, bq, Wk, bk, Wv, bv, Wo, bo):
    query_input = np.asarray(query_input, np.float32)
    key_value_input = np.asarray(key_value_input, np.float32)
    key_padding_mask = np.asarray(key_padding_mask)
    Wq = np.asarray(Wq, np.float32); bq = np.asarray(bq, np.float32)
    Wk = np.asarray(Wk, np.float32); bk = np.asarray(bk, np.float32)
    Wv = np.asarray(Wv, np.float32); bv = np.asarray(bv, np.float32)
    Wo = np.asarray(Wo, np.float32); bo = np.asarray(bo, np.float32)

    nc = _get_nc()
    in_maps = _shard_inputs(query_input, key_value_input, key_padding_mask,
                            Wq, bq, Wk, bk, Wv, bv, Wo, bo)
    res = run_bass_kernel_spmd(nc, in_maps, core_ids=list(range(NCORES)))

    # unshard: sum the 4 row-parallel partials per batch; V-bias contributes a
    # constant row (softmax rows sum to 1) folded in with bo here.
    const_row = (bv.astype(np.float64) @ Wo.astype(np.float64)) + bo.astype(np.float64)
    gpc = NCORES // B
    out = np.empty((B, SQ, D), np.float32)
    for b in range(B):
        acc = np.zeros((SQ, D), np.float64)
        for hg in range(gpc):
            acc += res.results[b * gpc + hg]["out_p"].astype(np.float64)
        out[b] = (acc + const_row[None, :]).astype(np.float32)
    return out



# revision 2
# speedup vs baseline: 1.6534x; 1.6534x over previous
"""Multihead attention (B=2, S=2048, D=1024, 16 heads) on 8 trn2 NeuronCores.

Sharding: data-parallel over batch (2 groups of 4 cores), tensor-parallel over
heads within a group (4 heads/core, W_q/W_k/W_v column-sliced, W_o row-sliced).
Each core returns a partial [2048, 1024] output; the host sums the 4 partials
per batch and adds the constant row bv @ Wo + bo (the V-bias contribution is
constant because softmax rows sum to 1).

v2 redesign driven by HW microbenchmarks:
  * kv compaction: the key-padding mask drops ~half the keys.  The host
    gathers the unmasked kv rows and pads to a multiple of 256, so scores /
    exp / PV / K,V-projections all shrink by ~40%.  Pad slots have zero K/V
    columns and a zero entry in the V mask column, so they contribute exactly
    nothing to either the numerator or the softmax denominator.
  * K=128 scores: matmuls with a 64-row contraction run at half rate on trn2
    silicon regardless of dtype.  K^T is stored per-head zero-padded to the
    full 128 partitions (the other 64 rows are zero), and Q^T keeps both
    sibling heads stacked; the zero rows annihilate the sibling head's
    contribution, so each score matmul contracts over all 128 partitions at
    full rate.
  * bf16 operands everywhere (PSUM accumulation stays fp32): same PE rate as
    float32r but half the DMA and SBUF traffic.
  * out_proj for q-chunk c is issued between the scores and PV of the first
    head of chunk c+1, hiding the normalization latency of the last head.
"""

import numpy as np
import ml_dtypes

import concourse.bacc as bacc
import concourse.tile as tile
import concourse.mybir as mybir
from concourse.bass_utils import run_bass_kernel_spmd

F32 = mybir.dt.float32
BF16 = mybir.dt.bfloat16
EXP = mybir.ActivationFunctionType.Exp
NPBF = ml_dtypes.bfloat16

B, SQ, SKV = 2, 2048, 2048
D, NH, HD = 1024, 16, 64
NCORES = 8
HPC = NH // (NCORES // B)     # 4 heads per core
CS = HPC * HD                 # 256 projection columns per core
QC = 512                      # q chunk
NQC = SQ // QC                # 4 q chunks
NDT = D // 128                # 8 contraction tiles
NST = SQ // 128               # 16 output row tiles


def _build(nkt: int, loop_n: int = 1):
    """nkt = number of 128-row kv tiles after compaction (even)."""
    kvc = nkt * 128
    nc = bacc.Bacc(None, target_bir_lowering=False)
    xT = nc.dram_tensor("xT", [D, SQ], BF16, kind="ExternalInput")
    kvT = nc.dram_tensor("kvT", [D, kvc], BF16, kind="ExternalInput")
    wq = nc.dram_tensor("wq", [128, NDT, CS], BF16, kind="ExternalInput")
    wk = nc.dram_tensor("wk", [128, NDT, CS], BF16, kind="ExternalInput")
    wv = nc.dram_tensor("wv", [128, NDT, CS], BF16, kind="ExternalInput")
    wo = nc.dram_tensor("wo", [128, 2, D], BF16, kind="ExternalInput")
    bqk = nc.dram_tensor("bqk", [128, 4], F32, kind="ExternalInput")
    mones = nc.dram_tensor("mones", [128, nkt, HPC], BF16, kind="ExternalInput")
    out_p = nc.dram_tensor("out_p", [SQ, D], F32, kind="ExternalOutput")

    # kv chunking for the K^T projection psum tiles (<=512 columns each)
    kch = []
    off = 0
    while off < kvc:
        w = min(512, kvc - off)
        kch.append((off, w))
        off += w

    with tile.TileContext(nc) as tc:
        with tc.tile_pool(name="const", bufs=1) as const, \
             tc.tile_pool(name="big", bufs=1) as big:
            wq_sb = const.tile([128, NDT, CS], BF16)
            wk_sb = const.tile([128, NDT, CS], BF16)
            wv_sb = const.tile([128, NDT, CS], BF16)
            wo_sb = const.tile([128, 2, D], BF16)
            bqk_sb = const.tile([128, 4], F32)
            nc.gpsimd.dma_start(out=wq_sb, in_=wq[:, :, :])
            nc.gpsimd.dma_start(out=bqk_sb, in_=bqk[:, :])
            nc.gpsimd.dma_start(out=wk_sb, in_=wk[:, :, :])
            nc.gpsimd.dma_start(out=wv_sb, in_=wv[:, :, :])
            nc.gpsimd.dma_start(out=wo_sb, in_=wo[:, :, :])

            QT = big.tile([128, 2, SQ], BF16)            # [hd(2x64), mh, q]
            KTp = big.tile([128, HPC, kvc], BF16)        # per-head, zero-padded
            V = big.tile([128, nkt, HPC, HD + 1], BF16)  # V + mask col at 64
            OT = big.tile([128, 2, SQ], BF16)            # [c(2x128), q]

            # zero the pad quarters of KTp once; the data quarters are
            # rewritten every iteration
            nc.vector.memset(KTp, 0.0)

            if loop_n > 1:
                loop_cm = tc.For_i(0, loop_n, 1)
                loop_cm.__enter__()

            nc.gpsimd.dma_start(out=V[:, :, :, HD:HD + 1], in_=mones[:, :, :])

            # ---- Phase 1: projections ----
            with tc.tile_pool(name="xin", bufs=3) as xin, \
                 tc.tile_pool(name="kvin", bufs=1) as kvin:
                kvts = []
                for dt in range(NDT):
                    kvt_t = kvin.tile([128, kvc], BF16, tag=f"kv{dt}", name=f"kvt{dt}")
                    nc.scalar.dma_start(out=kvt_t, in_=kvT[dt * 128:(dt + 1) * 128, :])
                    kvts.append(kvt_t)

                with tc.tile_pool(name="pqk", bufs=1, space="PSUM") as pqk:
                    # Q^T: dq-tile-outer accumulation into 8 resident psum banks
                    psq = [pqk.tile([128, QC], F32, tag=f"pq{i}", name=f"psq{i}")
                           for i in range(8)]
                    for dt in range(NDT):
                        xt_t = xin.tile([128, SQ], BF16, tag="xt", name=f"xt{dt}")
                        nc.sync.dma_start(out=xt_t, in_=xT[dt * 128:(dt + 1) * 128, :])
                        for i in range(8):
                            mh, qc = i // NQC, i % NQC
                            nc.tensor.matmul(psq[i],
                                             wq_sb[:, dt, mh * 128:(mh + 1) * 128],
                                             xt_t[:, qc * QC:(qc + 1) * QC],
                                             start=(dt == 0), stop=(dt == NDT - 1))
                    for i in range(8):
                        mh, qc = i // NQC, i % NQC
                        nc.vector.tensor_scalar_add(out=QT[:, mh, qc * QC:(qc + 1) * QC],
                                                    in0=psq[i], scalar1=bqk_sb[:, mh:mh + 1])
                    # K^T into per-head zero-padded layout
                    psk = {}
                    for mh in range(2):
                        for ci, (co, cw) in enumerate(kch):
                            i = mh * len(kch) + ci
                            ps = pqk.tile([128, QC], F32, tag=f"pq{i}",
                                          name=f"psk{i}")
                            psk[(mh, ci)] = ps
                            for dt in range(NDT):
                                nc.tensor.matmul(ps[:, 0:cw],
                                                 wk_sb[:, dt, mh * 128:(mh + 1) * 128],
                                                 kvts[dt][:, co:co + cw],
                                                 start=(dt == 0), stop=(dt == NDT - 1))
                    for mh in range(2):
                        for ci, (co, cw) in enumerate(kch):
                            ps = psk[(mh, ci)]
                            for po in range(2):
                                nc.vector.tensor_scalar_add(
                                    out=KTp[po * 64:(po + 1) * 64, 2 * mh + po,
                                            co:co + cw],
                                    in0=ps[po * 64:(po + 1) * 64, 0:cw],
                                    scalar1=bqk_sb[po * 64:(po + 1) * 64,
                                                   2 + mh:3 + mh])

                with tc.tile_pool(name="pv", bufs=4, space="PSUM") as pv:
                    # V natural [kv, 256]; pad slots are zero because the
                    # compacted kvT columns there are zero
                    for t in range(nkt):
                        ps = pv.tile([128, CS], F32, tag="pv")
                        for dt in range(NDT):
                            nc.tensor.matmul(ps,
                                             kvts[dt][:, t * 128:(t + 1) * 128],
                                             wv_sb[:, dt, :],
                                             start=(dt == 0), stop=(dt == NDT - 1))
                        nc.vector.tensor_copy(
                            out=V[:, t, :, 0:HD],
                            in_=ps.rearrange("p (h d) -> p h d", h=HPC))

            # ---- Phase 2: attention with interleaved output projection ----
            with tc.tile_pool(name="pp", bufs=2) as pp, \
                 tc.tile_pool(name="outp", bufs=2) as outp, \
                 tc.tile_pool(name="small", bufs=2) as small, \
                 tc.tile_pool(name="psc", bufs=2, space="PSUM") as psc, \
                 tc.tile_pool(name="pso", bufs=2, space="PSUM") as pso, \
                 tc.tile_pool(name="pout", bufs=2, space="PSUM") as pout:

                def norm_store(po_, po, mh, qsl):
                    rec = small.tile([HD + 1, QC], F32, tag="rec", name="rec")
                    nc.vector.reciprocal(out=rec[HD:HD + 1, :], in_=po_[HD:HD + 1, :])
                    rec0 = small.tile([1, QC], F32, tag="rec0", name="rec0")
                    nc.sync.dma_start(out=rec0[0:1, :], in_=rec[HD:HD + 1, :])
                    rb = small.tile([HD, QC], F32, tag="rb", name="rb")
                    nc.gpsimd.partition_broadcast(rb, rec0[0:1, :])
                    ot_tmp = small.tile([HD, QC], BF16, tag="ott", name="ot_tmp")
                    nc.vector.tensor_mul(out=ot_tmp, in0=po_[0:HD, :], in1=rb)
                    nc.sync.dma_start(out=OT[po:po + 64, mh, qsl], in_=ot_tmp)

                def out_proj(qc):
                    for st in range(qc * NQC, (qc + 1) * NQC):
                        ot_sb = outp.tile([128, D], F32, tag="osb", name="ot_sb")
                        for nk in range(2):
                            ps = pout.tile([128, 512], F32, tag="po2", name="ps_out")
                            for ct in range(2):
                                nc.tensor.matmul(ps,
                                                 OT[:, ct, st * 128:(st + 1) * 128],
                                                 wo_sb[:, ct, nk * 512:(nk + 1) * 512],
                                                 start=(ct == 0), stop=(ct == 1))
                            nc.vector.tensor_copy(out=ot_sb[:, nk * 512:(nk + 1) * 512],
                                                  in_=ps)
                        nc.sync.dma_start(out=out_p[st * 128:(st + 1) * 128, :],
                                          in_=ot_sb)

                groups = [(t0, min(2, nkt - t0)) for t0 in range(0, nkt, 2)]

                for qc in range(NQC):
                    qsl = slice(qc * QC, (qc + 1) * QC)
                    for h in range(HPC):
                        mh, po = h // 2, (h % 2) * 64
                        P = pp.tile([128, nkt, QC], BF16, tag="P0", name="P")
                        for g0, gn in groups:
                            ss = psc.tile([128, 2, QC], F32, tag="ss", name="ss")
                            for tt in range(gn):
                                t = g0 + tt
                                nc.tensor.matmul(ss[:, tt, :],
                                                 KTp[:, h, t * 128:(t + 1) * 128],
                                                 QT[:, mh, qsl],
                                                 start=True, stop=True)
                            nc.scalar.activation(out=P[:, g0:g0 + gn, :],
                                                 in_=ss[:, 0:gn, :], func=EXP,
                                                 scale=0.125)
                        if h == 1 and qc > 0:
                            out_proj(qc - 1)
                        po_ = pso.tile([HD + 1, QC], F32, tag="po", name="po_")
                        for t in range(nkt):
                            nc.tensor.matmul(po_, V[:, t, h, :], P[:, t, :],
                                             start=(t == 0), stop=(t == nkt - 1))
                        norm_store(po_, po, mh, qsl)
                out_proj(NQC - 1)

            if loop_n > 1:
                loop_cm.__exit__(None, None, None)

    nc.compile()
    return nc


_NC_CACHE = {}


def _get_nc(nkt):
    if nkt not in _NC_CACHE:
        _NC_CACHE[nkt] = _build(nkt)
    return _NC_CACHE[nkt]


def _plan(key_padding_mask):
    """Compacted kv tile count (even, >=2) and per-batch keep indices."""
    keep_idx = [np.where(~np.asarray(key_padding_mask[b]))[0] for b in range(B)]
    kvc = max(max(len(ix) for ix in keep_idx), 1)
    kvc = ((kvc + 255) // 256) * 256
    return kvc // 128, keep_idx


def _shard_inputs(nkt, keep_idx, query_input, key_value_input, key_padding_mask,
                  Wq, bq, Wk, bk, Wv, bv, Wo, bo):
    kvc = nkt * 128
    in_maps = []
    xTb = [np.ascontiguousarray(query_input[b].T.astype(NPBF)) for b in range(B)]
    kvTb = []
    monesb = []
    for b in range(B):
        ix = keep_idx[b]
        kvT = np.zeros((D, kvc), NPBF)
        kvT[:, :len(ix)] = key_value_input[b][ix].T.astype(NPBF)
        kvTb.append(kvT)
        m01 = np.zeros((kvc,), NPBF)
        m01[:len(ix)] = 1
        mcol = np.ascontiguousarray(m01.reshape(nkt, 128).T)          # [128, nkt]
        monesb.append(np.ascontiguousarray(
            np.repeat(mcol[:, :, None], HPC, axis=2)))                # [128, nkt, HPC]
    for c in range(NCORES):
        b, hg = c // (NCORES // B), c % (NCORES // B)
        cs = slice(hg * CS, (hg + 1) * CS)
        in_maps.append({
            "xT": xTb[b],
            "kvT": kvTb[b],
            "wq": np.ascontiguousarray(
                Wq[:, cs].reshape(NDT, 128, CS).transpose(1, 0, 2).astype(NPBF)),
            "wk": np.ascontiguousarray(
                Wk[:, cs].reshape(NDT, 128, CS).transpose(1, 0, 2).astype(NPBF)),
            "wv": np.ascontiguousarray(
                Wv[:, cs].reshape(NDT, 128, CS).transpose(1, 0, 2).astype(NPBF)),
            "wo": np.ascontiguousarray(
                Wo[cs, :].reshape(2, 128, D).transpose(1, 0, 2).astype(NPBF)),
            "bqk": np.ascontiguousarray(
                np.stack([bq[cs][:128], bq[cs][128:], bk[cs][:128], bk[cs][128:]],
                         axis=1).astype(np.float32)),
            "mones": monesb[b],
        })
    return in_maps


def kernel(query_input, key_value_input, key_padding_mask,
           Wq, bq, Wk, bk, Wv, bv, Wo, bo):
    query_input = np.asarray(query_input, np.float32)
    key_value_input = np.asarray(key_value_input, np.float32)
    key_padding_mask = np.asarray(key_padding_mask)
    Wq = np.asarray(Wq, np.float32); bq = np.asarray(bq, np.float32)
    Wk = np.asarray(Wk, np.float32); bk = np.asarray(bk, np.float32)
    Wv = np.asarray(Wv, np.float32); bv = np.asarray(bv, np.float32)
    Wo = np.asarray(Wo, np.float32); bo = np.asarray(bo, np.float32)

    nkt, keep_idx = _plan(key_padding_mask)
    nc = _get_nc(nkt)
    in_maps = _shard_inputs(nkt, keep_idx, query_input, key_value_input,
                            key_padding_mask, Wq, bq, Wk, bk, Wv, bv, Wo, bo)
    res = run_bass_kernel_spmd(nc, in_maps, core_ids=list(range(NCORES)))

    # unshard: sum the 4 row-parallel partials per batch; V-bias contributes a
    # constant row (softmax rows sum to 1) folded in with bo here.
    const_row = (bv.astype(np.float64) @ Wo.astype(np.float64)) + bo.astype(np.float64)
    gpc = NCORES // B
    out = np.empty((B, SQ, D), np.float32)
    for b in range(B):
        acc = np.zeros((SQ, D), np.float64)
        for hg in range(gpc):
            acc += res.results[b * gpc + hg]["out_p"].astype(np.float64)
        out[b] = (acc + const_row[None, :]).astype(np.float32)
    return out
